# revision 29
# baseline (speedup 1.0000x reference)
"""InterleavedHeadAttention Trainium2 kernel.

Sharding (8 cores): core c handles batch b = c//4 and 4 output heads
[4*(c%4), 4*(c%4)+4).  The alpha head-mixing einsum is folded into the
QKV projection weights on the host.  Pseudo-head merge uses (p, n) flat
ordering (attention is permutation invariant; the token-causal mask
depends only on n).

Per-core engine split:
  PE:   fp8 DoubleRow Q/K projections (host-quantized x and W, x64
        scale), bf16 V projection, fp8 scores (both pq halves in one
        matmul via packed [64,(pq,n)] q/k layout), bf16 AV with
        ones-augmented V (softmax denominator comes out as row 64),
        recip-broadcast, per-head output projection partials.
  Act:  exp over both pq halves of each score tile in one instruction.
  DVE:  q-side PSUM->SBUF copies with fused bias add, reciprocal,
        tri-mask muls, final normalize muls.
  Pool: k-side copies, vaug copies, output staging copies, memsets.
  SP:   all DMA issue.

Host folds: alpha into Wq/Wk/Wv, collapse into Wo, x64 fp8 scaling into
Wq/Wk and bq/bk (exp scale compensates by 1/4096).  Partial (S, HID)
f32 outputs are summed on host (+bo).
"""
import numpy as np
import ml_dtypes

import concourse.bacc as bacc
import concourse.bass as bass
import concourse.tile as tile
import concourse.mybir as mybir
from concourse.bass_utils import run_bass_kernel_spmd

B, S, HID, H, P = 2, 1024, 1024, 16, 2
D = HID // H          # 64
HL = 4                # heads per core
G = HL * P            # (h,pk) groups per core = 8
HPD = HL * P * D      # 512 projection rows per core
BF = mybir.dt.bfloat16
F32 = mybir.dt.float32
F8 = mybir.dt.float8e4
NCORES = 8
QSC = 64.0            # fp8 prescale folded into Wq/Wk on host
EXP_SCALE = 0.125 / (QSC * QSC)

_compiled = None


def _build():
    nc = bacc.Bacc()
    x8 = nc.dram_tensor("x8", (128, 8192), F8, kind="ExternalInput")
    w8q = nc.dram_tensor("w8q", (128, 4096), F8, kind="ExternalInput")
    w8k = nc.dram_tensor("w8k", (128, 4096), F8, kind="ExternalInput")
    bq8 = nc.dram_tensor("bq8", (128, 4), F32, kind="ExternalInput")
    bk8 = nc.dram_tensor("bk8", (128, 4), F32, kind="ExternalInput")
    xT = nc.dram_tensor("xT", (128, 8192), BF, kind="ExternalInput")
    wv = nc.dram_tensor("wv", (128, 4096), BF, kind="ExternalInput")
    bv = nc.dram_tensor("bv", (1, HPD), BF, kind="ExternalInput")
    wo = nc.dram_tensor("wo", (128, HL * HID), BF, kind="ExternalInput")
    tri = nc.dram_tensor("tri", (128, 256), BF, kind="ExternalInput")
    out = nc.dram_tensor("o", (S, HID), F32, kind="ExternalOutput")

    KT = HID // 128   # 8 hid tiles (bf16) / 4 DR pairs (fp8)
    NT = S // 512     # 2 n windows per p
    DR = mybir.MatmulPerfMode.DoubleRow
    with tile.TileContext(nc) as tc:
        with tc.tile_pool(name="persist", bufs=1) as pp, \
             tc.tile_pool(name="ppool", bufs=40) as ppl, \
             tc.tile_pool(name="small", bufs=4) as sml, \
             tc.tile_pool(name="osb", bufs=3) as osb, \
             tc.tile_pool(name="psc", bufs=2, space=bass.MemorySpace.PSUM) as psc, \
             tc.tile_pool(name="psav", bufs=3, space=bass.MemorySpace.PSUM) as psav, \
             tc.tile_pool(name="psmm", bufs=1, space=bass.MemorySpace.PSUM) as psmm:

            # ---- persistent inputs (few big DMAs, QK inputs first) ----
            ones = pp.tile([1, 512], BF, tag="ones", name="ones")
            nc.gpsimd.memset(ones[:], 1.0)

            x8_all = pp.tile([128, 8192], F8, tag="x8", name="x8s")
            nc.sync.dma_start(x8_all[:], x8[:])
            x8_sb = [x8_all[:, r * 2048:(r + 1) * 2048] for r in range(4)]
            w8_sb = {}
            for nm, dram in (("q", w8q), ("k", w8k)):
                wa = pp.tile([128, 4096], F8, tag=f"w8{nm}", name=f"w8{nm}s")
                nc.sync.dma_start(wa[:], dram[:])
                w8_sb[nm] = [wa[:, r * 1024:(r + 1) * 1024] for r in range(4)]
            b8_sb = {}
            for nm, dram in (("q", bq8), ("k", bk8)):
                b8_sb[nm] = pp.tile([128, 4], F32, tag=f"b8{nm}", name=f"b8{nm}")
                nc.sync.dma_start(b8_sb[nm][:], dram[:])
            tri_sb = pp.tile([128, 256], BF, tag="tri", name="tri")
            nc.sync.dma_start(tri_sb[:], tri[:])
            # xT/wv/woe DMAs are issued later (between qkproj emissions) so
            # the small qdr/kdr rearrange transfers aren't queued behind them
            # on the serial DMA engines.
            xt_all = pp.tile([128, KT * S], BF, tag="xt", name="xts")
            xt_sb = [xt_all[:, k * S:(k + 1) * S] for k in range(KT)]
            wv_all = pp.tile([128, KT * HPD], BF, tag="wv", name="wvs")
            wv_sb = [wv_all[:, k * HPD:(k + 1) * HPD] for k in range(KT)]
            bv_sb = pp.tile([1, HPD], BF, tag="bv", name="bv")
            woe_all = pp.tile([128, HL * HID], BF, tag="woe", name="woes")
            woe_sb = [woe_all[:, h * HID:(h + 1) * HID] for h in range(HL)]

            def load_v_inputs():
                nc.sync.dma_start(xt_all[:], xT[:])
                nc.sync.dma_start(wv_all[:], wv[:])
                nc.sync.dma_start(bv_sb[:], bv[:])

            def load_wo():
                nc.sync.dma_start(woe_all[:], wo[:])

            # ---- persistent intermediates ----
            # q/k packed [64 d, (half 2, n 1024)] fp8, values x64
            qtp = [pp.tile([64, 2048], F8, tag=f"qtp{h}", name=f"qtp{h}")
                   for h in range(HL)]
            ktp = [pp.tile([64, 2048], F8, tag=f"ktp{h}", name=f"ktp{h}")
                   for h in range(HL)]
            vaug = [pp.tile([128, G * 65], BF, tag=f"va{j}", name=f"va{j}")
                    for j in range(S // 128)]
            ot2 = [pp.tile([128, S], BF, tag=f"ot2{h}", name=f"ot2{h}")
                   for h in range(HL)]

            def vproj(jt):
                v3 = vaug[jt].rearrange("p (g e) -> p g e", e=65)
                nc.gpsimd.memset(v3[:, :, 64:65], 1.0)
                acc = psmm.tile([128, 512], F32, tag="mm", name="mm")
                for k in range(KT):
                    nc.tensor.matmul(
                        acc[:], xt_sb[k][:, jt * 128:(jt + 1) * 128],
                        wv_sb[k][:], start=(k == 0), stop=False)
                nc.tensor.matmul(acc[:], ones[:, 0:128], bv_sb[:],
                                 start=False, stop=True)
                nc.vector.tensor_copy(
                    v3[:, :, 0:64], acc[:].rearrange("p (g e) -> p g e", e=64))

            # DR-interleaved q/k for the In=1 scores: q only needs the
            # n>=512 query half -> qdr [32, (dh 2, pq 2, 512)]; k needs all
            # keys -> kdr [32, (dh 2, pk 2, 1024)]
            qdr = [pp.tile([32, 2048], F8, tag=f"qdr{h}", name=f"qdr{h}")
                   for h in range(HL)]
            kdr = [pp.tile([32, 4096], F8, tag=f"kdr{h}", name=f"kdr{h}")
                   for h in range(HL)]

            def qkproj(mt, early=False, mid=None):
                # nt-outer so the In=0 half (queries/keys 0-511) lands first;
                # mid() is emitted between the halves.
                for nt in range(NT):
                    for nm in ("q", "k"):
                        dst = qtp[mt] if nm == "q" else ktp[mt]
                        eng = nc.vector
                        pool = psav if (early and (nt + (nm == "k")) % 2) \
                            else psmm
                        tg = "av" if pool is psav else "mm"
                        acc = pool.tile([128, 512], F32, tag=tg, name="mm")
                        for r in range(4):
                            w3 = w8_sb[nm][r].rearrange("p (i m) -> p i m", i=2)
                            xx = x8_sb[r].rearrange("p (i n) -> p i n", i=2)
                            nc.tensor.matmul(
                                acc[:], w3[:, :, mt * 128:(mt + 1) * 128],
                                xx[:, :, nt * 512:(nt + 1) * 512],
                                start=(r == 0), stop=(r == 3), perf_mode=DR)
                        bb = b8_sb[nm]
                        for half in range(2):
                            eng.tensor_scalar_add(
                                dst[:, half * 1024 + nt * 512:
                                    half * 1024 + (nt + 1) * 512],
                                acc[half * 64:(half + 1) * 64, :],
                                bb[half * 64:(half + 1) * 64, mt:mt + 1])
                    if nt == 0 and mid is not None:
                        mid()
                for dh in range(2):
                    nc.sync.dma_start(
                        qdr[mt][:, dh * 1024:(dh + 1) * 1024]
                        .rearrange("p (i n) -> p i n", i=2),
                        qtp[mt][dh * 32:(dh + 1) * 32, :]
                        .rearrange("p (i n) -> p i n", i=2)[:, :, 512:1024])
                    nc.sync.dma_start(
                        kdr[mt][:, dh * 2048:(dh + 1) * 2048],
                        ktp[mt][dh * 32:(dh + 1) * 32, :])

            # software-pipelined attention: scores+exp emitted ahead of the
            # AV phase so the Act engine never starves; pt tiles carry the
            # probabilities between the two phases.
            pts = {}

            def attn_scores(h, In, dr=True, jns=None):
                q3 = qdr[h].rearrange("p (i n) -> p i n", i=2)
                k3 = kdr[h].rearrange("p (i n) -> p i n", i=2)
                tiles = pts.setdefault((h, In), [])
                for Jn in (jns if jns is not None else range(4 * In + 4)):
                    FF = 128 * (Jn - 4 * In)
                    c0 = FF if FF >= 0 else 0
                    for pk in range(2):
                        sc = psc.tile([128, 1024], F32, tag="sc", name="sc")
                        s3 = sc.rearrange("p (i n) -> p i n", i=2)
                        for pq in range(2):
                            if dr:
                                nc.tensor.matmul(
                                    s3[:, pq, c0:512],
                                    k3[:, :, pk * 1024 + Jn * 128:
                                       pk * 1024 + (Jn + 1) * 128],
                                    q3[:, :, pq * 512 + c0:(pq + 1) * 512],
                                    start=True, stop=True, perf_mode=DR)
                            else:
                                # non-DR path skips the qdr/kdr rearrange
                                # round-trip: used for the first window so
                                # the Act engine starts ~2.5us earlier
                                nc.tensor.matmul(
                                    s3[:, pq, c0:512],
                                    ktp[h][:, pk * 1024 + Jn * 128:
                                           pk * 1024 + (Jn + 1) * 128],
                                    qtp[h][:, pq * 1024 + In * 512 + c0:
                                           pq * 1024 + (In + 1) * 512],
                                    start=True, stop=True)
                        pt = ppl.tile([128, 1024], BF, tag="p", name="p")
                        p3 = pt.rearrange("p (i n) -> p i n", i=2)
                        nc.scalar.activation(
                            p3[:, :, c0:512], s3[:, :, c0:512],
                            mybir.ActivationFunctionType.Exp, scale=EXP_SCALE)
                        if FF >= 0:
                            nc.gpsimd.tensor_mul(
                                p3[:, :, c0:c0 + 128], p3[:, :, c0:c0 + 128],
                                tri_sb[:].rearrange("p (i c) -> p i c", i=2))
                        tiles.append((p3, c0))

            def attn_av(h, In):
                tiles = pts.pop((h, In))
                avp = [psav.tile([65, 512], F32, tag="av", name="av")
                       for _ in range(2)]
                u = 0
                for Jn in range(4 * In + 4):
                    for pk in range(2):
                        p3, c0 = tiles[u]
                        u += 1
                        g = h * 2 + pk
                        for pq in range(2):
                            nc.tensor.matmul(
                                avp[pq][:, c0:512],
                                vaug[Jn][:, g * 65:g * 65 + 65],
                                p3[:, pq, c0:512],
                                start=(Jn == 0 and pk == 0),
                                stop=(Jn == 4 * In + 3 and pk == 1))
                for pq in range(2):
                    recip = sml.tile([1, 512], BF, tag="recip", name="recip")
                    with nc.allow_low_precision(reason="softmax recip bf16"):
                        nc.vector.reciprocal(recip[:], avp[pq][64:65, :])
                    bcs = sml.tile([64, 512], BF, tag="bcs", name="bcs")
                    nc.gpsimd.partition_broadcast(bcs[:], recip[:])
                    nc.vector.tensor_mul(
                        ot2[h][pq * 64:(pq + 1) * 64, In * 512:(In + 1) * 512],
                        avp[pq][0:64, :], bcs[:])

            def outproj(mts, pools, act_copy=False):
                for i, mt in enumerate(mts):
                    for jt in range(HID // 512):
                        pool, tg = pools[(i * 2 + jt) % len(pools)]
                        op = pool.tile([128, 512], F32, tag=tg, name="op")
                        for h in range(HL):
                            nc.tensor.matmul(
                                op[:], ot2[h][:, mt * 128:(mt + 1) * 128],
                                woe_sb[h][:, jt * 512:(jt + 1) * 512],
                                start=(h == 0), stop=(h == HL - 1))
                        ob = osb.tile([128, 512], F32, tag="ob", name="ob")
                        if act_copy and jt % 2:
                            nc.scalar.copy(ob[:], op[:])
                        else:
                            nc.vector.tensor_copy(ob[:], op[:])
                        nc.sync.dma_start(
                            out[mt * 128:(mt + 1) * 128,
                                jt * 512:(jt + 1) * 512], ob[:])

            qkproj(0, early=True,
                   mid=lambda: (load_v_inputs(),
                                attn_scores(0, 0, dr=False)))
            qkproj(1, early=True,
                   mid=lambda: attn_scores(1, 0, dr=False))
            qkproj(2, early=True,
                   mid=lambda: attn_scores(2, 0, dr=False))
            qkproj(3, early=True,
                   mid=lambda: attn_scores(3, 0, dr=False))
            load_wo()
            for jt in range(4):
                vproj(jt)
            attn_scores(0, 1, jns=range(0, 4))
            attn_av(0, 0)
            attn_scores(0, 1, jns=range(4, 8))
            attn_av(1, 0)
            attn_scores(1, 1, jns=range(0, 4))
            attn_av(2, 0)
            attn_scores(1, 1, jns=range(4, 8))
            attn_av(3, 0)
            for jt in range(4, 8):
                vproj(jt)
            attn_av(0, 1)
            attn_scores(2, 1, jns=range(0, 4))
            attn_av(1, 1)
            attn_scores(2, 1, jns=range(4, 8))
            outproj(range(0, 4), [(psmm, "mm")])
            attn_scores(3, 1, jns=range(0, 4))
            attn_av(2, 1)
            attn_scores(3, 1, jns=range(4, 8))
            attn_av(3, 1)
            outproj(range(4, 8), [(psc, "sc"), (psmm, "mm")], act_copy=True)
    nc.compile()
    return nc


def _prep(inputs):
    bf = ml_dtypes.bfloat16
    f8 = ml_dtypes.float8_e4m3
    hs = np.asarray(inputs["hidden_states"], np.float32)
    maps = []
    # tri2[r, i*128+c] = 1 iff c >= r, both pq halves
    tri1 = np.triu(np.ones((128, 128), np.float32))
    tri2 = np.concatenate([tri1, tri1], axis=1).astype(bf)
    eff = {}
    for nm in ("q", "k", "v"):
        W = np.asarray(inputs[f"W{nm}"], np.float32)
        bb = np.asarray(inputs[f"b{nm}"], np.float32)
        al = np.asarray(inputs[f"alpha_{nm}"], np.float32)
        We = np.einsum("mhp,mdc->hpdc", al, W.reshape(H, D, HID))
        be = np.einsum("mhp,md->hpd", al, bb.reshape(H, D))
        eff[nm] = (We, be)
    Wo = np.asarray(inputs["Wo"], np.float32)
    col = np.asarray(inputs["collapse"], np.float32)
    Woe = np.einsum("hp,jhd->hpdj", col, Wo.reshape(HID, H, D))  # (H,P,D,HID)
    for c in range(NCORES):
        b, gidx = c // 4, c % 4
        hs_sl = slice(gidx * HL, (gidx + 1) * HL)
        xTb = np.ascontiguousarray(hs[b].T)              # (hid, n)
        # SBUF-ready layouts: partition p first, 128 contiguous descriptors.
        # x8[p, r*2048 + i*1024 + n] = x[(2r+i)*128+p, n]  (fp8 DR pairs)
        x8v = (xTb.reshape(4, 2, 128, S).transpose(2, 0, 1, 3)
               .reshape(128, 8192))
        # xT[p, r*1024 + n] = x[r*128+p, n]
        xtv = xTb.reshape(8, 128, S).transpose(1, 0, 2).reshape(128, 8192)
        m = {"x8": np.ascontiguousarray(x8v).astype(f8),
             "xT": np.ascontiguousarray(xtv).astype(bf),
             "tri": tri2}
        for nm in ("q", "k"):
            We, be = eff[nm]
            Wsl = We[hs_sl].reshape(HPD, HID) * QSC       # (m, hid)
            # w8[p, r*1024 + i*512 + mm] = Wsl[mm, (2r+i)*128+p]
            w8v = (Wsl.T.reshape(4, 2, 128, HPD).transpose(2, 0, 1, 3)
                   .reshape(128, 4096))
            m[f"w8{nm}"] = np.ascontiguousarray(w8v).astype(f8)
            m[f"b{nm}8"] = np.ascontiguousarray(
                (be[hs_sl].reshape(4, 128) * QSC).T).astype(np.float32)
        Wev, bev = eff["v"]
        Wslv = Wev[hs_sl].reshape(HPD, HID)
        # wv[p, r*512 + mm] = Wslv[mm, r*128+p]
        wvv = (Wslv.T.reshape(8, 128, HPD).transpose(1, 0, 2)
               .reshape(128, 4096))
        m["wv"] = np.ascontiguousarray(wvv).astype(bf)
        m["bv"] = bev[hs_sl].reshape(1, HPD).astype(bf)
        # wo[p, h*HID + j] = Woe[h, pd=p, j]
        wov = (Woe[hs_sl].reshape(HL, P * D, HID).transpose(1, 0, 2)
               .reshape(128, HL * HID))
        m["wo"] = np.ascontiguousarray(wov).astype(bf)
        maps.append(m)
    return maps


def kernel(**inputs):
    global _compiled
    if _compiled is None:
        _compiled = _build()
    maps = _prep(inputs)
    res = run_bass_kernel_spmd(_compiled, maps, core_ids=list(range(NCORES)))
    bo = np.asarray(inputs["bo"], np.float32)
    out = np.zeros((B, S, HID), np.float32)
    for c in range(NCORES):
        out[c // 4] += res.results[c]["o"]
    out += bo
    return out
